# revision 2
# baseline (speedup 1.0000x reference)
"""CRF NLL loss kernel v4 for Trainium2 (8 NeuronCores, batch-parallel).

Same segmented-forward algorithm as v3 (H=32 concurrent segment chains,
rank-1 Perron-Frobenius composition, warmup W=6, off-chain gold), with
engine balancing:
  - group 0: 2 matmuls [296|296] -> 1 DVE mul [126,592]
  - group 1: 2 matmuls [444|148] -> DVE mul [126,444] + (ACT psum->sbuf
    copy [126,148] -> Pool mul) -- carves 148 elems/tick off DVE, the
    bottleneck engine, onto ACT/Pool spare cycles.
  - chunk 0 ships col-major and is DMA'd/exp'd in 4 column slices so the
    first tick starts ~4us earlier.
  - the gold add-tree (Pool) is emitted one instruction per tick inside
    the chain loop, hiding it entirely in Pool's idle time.
"""
import os
import sys

import numpy as np

sys.path.insert(0, "/opt/trn_rl_repo")

from contextlib import ExitStack

import concourse.bacc as bacc
import concourse.bass as bass
import concourse.tile as tile
from concourse import mybir
from concourse.bass_utils import run_bass_kernel_spmd

# problem constants (hardcoded per spec)
B, T, K = 4096, 2048, 11
START, STOP = 10, 9
NCORES = 8
BL = B // NCORES          # 512 sentences per core
G, KT, J = 14, 9, 37
P = 128
PL = G * KT
H = 32                    # time segments
L = T // H                # 64
W = 1                     # warmup ticks
NT = W + L                # 70 ticks
TC = 4                    # ticks per emission chunk
NCH = L // TC             # 16 chunks
NWIN = H                  # 32 windows per chunk (no tail needed at W=1)
NGRP = 2
SPG = H // NGRP           # 16
SW = SPG * J              # 592
HB = 8 * J                # 296 (group-0 psum half)
CA = 12 * J               # 444 (group-1 bank A)
CB = 4 * J                # 148 (group-1 bank B -> ACT/Pool carve)
C0 = 3.25                 # fp8(e4m3)-exact recentering constant
GT = 512                  # gold values per partition-lane
NGC = 4
GC = GT // NGC            # 128
GOLD_T0 = 32              # first tick that may emit a gold thunk

F32 = mybir.dt.float32
BF16 = mybir.dt.bfloat16
F8 = mybir.dt.float8e4

CHUNK_ELEMS = NWIN * TC * J


def _build_nc():
    nc = bacc.Bacc()
    e_in = nc.declare_dram_parameter(
        "emis8", [P, NCH, CHUNK_ELEMS], F8, isOutput=False)
    gold_in = nc.declare_dram_parameter(
        "gold8", [P, NGC, J * GC], F8, isOutput=False)
    bd_in = nc.declare_dram_parameter("bd_lhst", [P, P], BF16, isOutput=False)
    ones_in = nc.declare_dram_parameter("ones_bd", [P, G], BF16, isOutput=False)
    astop_in = nc.declare_dram_parameter("astop_bd", [P, G], BF16,
                                         isOutput=False)
    init_in = nc.declare_dram_parameter("init_st", [P, NGRP * SW], BF16,
                                        isOutput=False)
    onesf_in = nc.declare_dram_parameter("ones_f32", [P, G], F32,
                                         isOutput=False)
    out_ext = nc.declare_dram_parameter("nll", [G, J], F32, isOutput=True)

    with tile.TileContext(nc) as tc, ExitStack() as ctx:
        consts = ctx.enter_context(tc.tile_pool(name="consts", bufs=1))
        epool = ctx.enter_context(tc.tile_pool(name="epool", bufs=1))
        raw_pool = ctx.enter_context(tc.tile_pool(name="raw", bufs=2))
        state_pool = ctx.enter_context(tc.tile_pool(name="state", bufs=3))
        small_pool = ctx.enter_context(tc.tile_pool(name="small", bufs=3))
        psum_pool = ctx.enter_context(
            tc.tile_pool(name="psum", bufs=1, space="PSUM"))
        cs_pool = ctx.enter_context(
            tc.tile_pool(name="cpsum", bufs=1, space="PSUM"))

        bias_c0 = consts.tile([P, 1], F32)
        nc.vector.memset(bias_c0, -C0)
        warm = consts.tile([P, 1], F32)
        nc.scalar.activation(
            out=warm, in_=bias_c0, func=mybir.ActivationFunctionType.Exp,
            bias=0.0, scale=1.0)

        # chunk 0 ships col-major [TC, NWIN, J]; DMA+exp in 4 col slices
        echunks = [None] * NCH
        raw0 = raw_pool.tile([P, TC, NWIN, J], F8, tag="raw0")
        ech0 = epool.tile([P, NWIN, TC, J], F8, tag="ech0")
        nwj = NWIN * J
        for c in range(TC):
            nc.sync.dma_start(out=raw0[:, c], in_=e_in[:, 0, c * nwj:(c + 1) * nwj])
            nc.scalar.activation(
                out=ech0[:, :, c, :], in_=raw0[:, c],
                func=mybir.ActivationFunctionType.Exp, bias=bias_c0, scale=1.0)
        echunks[0] = ech0

        bd = consts.tile([P, P], BF16)
        nc.sync.dma_start(out=bd, in_=bd_in[:])
        init_st = consts.tile([P, NGRP * SW], BF16)
        nc.sync.dma_start(out=init_st, in_=init_in[:])
        ones_bd = consts.tile([P, G], BF16)
        nc.sync.dma_start(out=ones_bd, in_=ones_in[:])
        astop_bd = consts.tile([P, G], BF16)
        nc.sync.dma_start(out=astop_bd, in_=astop_in[:])
        ones_f32 = consts.tile([P, G], F32)
        nc.sync.dma_start(out=ones_f32, in_=onesf_in[:])

        states = []
        for g in range(NGRP):
            st = state_pool.tile([P, SW], BF16, tag=f"st{g}")
            nc.gpsimd.tensor_copy(out=st, in_=init_st[:, g * SW:(g + 1) * SW])
            states.append(st)

        for k in range(1, NCH):
            raw = raw_pool.tile([P, NWIN, TC, J], F8, tag="raw")
            nc.sync.dma_start(out=raw, in_=e_in[:, k, :])
            ech = epool.tile([P, NWIN, TC, J], F8, tag=f"ech{k}")
            nc.scalar.activation(
                out=ech, in_=raw, func=mybir.ActivationFunctionType.Exp,
                bias=bias_c0, scale=1.0)
            echunks[k] = ech

        # gold: DMAs up front, Pool add-tree woven into the tick loop
        gacc = consts.tile([P, J], F32)
        nc.vector.memset(gacc, 0.0)
        glvl_a = consts.tile([P, J, GC // 2], F32)
        glvl_b = consts.tile([P, J, GC // 4], F32)
        glvl = [glvl_a, glvl_b]
        gold_thunks = []

        def make_gold_chunk(k):
            def dma():
                graw = raw_pool.tile([P, J, GC], F8, tag="graw")
                nc.sync.dma_start(out=graw, in_=gold_in[:, k, :])
                make_gold_chunk.cur = graw
            gold_thunks.append(dma)
            state = {"n": GC, "li": 0}

            def level(state=state):
                src = make_gold_chunk.cur
                n, li = state["n"], state["li"]
                half = n // 2
                if half >= 1 and n > 1:
                    dst = glvl[li % 2]
                    nc.gpsimd.tensor_add(
                        out=dst[:, :, :half], in0=src[:, :, :half],
                        in1=src[:, :, half:n])
                    make_gold_chunk.cur = dst
                    state["n"], state["li"] = half, li + 1
                    if half == 1:
                        nc.gpsimd.tensor_add(
                            out=gacc, in0=gacc,
                            in1=make_gold_chunk.cur[:, :, 0])
            for _ in range(7):
                gold_thunks.append(level)
        for k in range(NGC):
            make_gold_chunk(k)

        y_ln = consts.tile([G, H, J], F32)
        w_ln = consts.tile([G, H, J], F32)

        def colsums():
            cs = cs_pool.tile([G, 4, 512], F32, tag="ycs")
            for g in range(NGRP):
                for h in range(2):
                    nc.tensor.matmul(
                        cs[:, 2 * g + h, :HB], ones_bd,
                        states[g][:, h * HB:(h + 1) * HB],
                        start=True, stop=True)
            return cs

        for tau in range(NT):
            if tau < L:
                ech, wb, col = echunks[tau // TC], 0, tau % TC
            else:
                ech, wb, col = echunks[(tau - L) // TC], 1, (tau - L) % TC
            new_states = []
            for g in range(NGRP):
                st = states[g]
                stn = state_pool.tile([P, SW], BF16, tag=f"st{g}")
                ps = psum_pool.tile([P, 2, 512], F32, tag=f"ps{g}")
                w0 = wb + SPG * g
                if g == 0 and tau < W:
                    # chain-0 warmup: segment 0 (cols [0:J)) skips the matmul
                    nc.tensor.matmul(ps[:, 0, J:HB], bd, st[:, J:HB],
                                     start=True, stop=True)
                    nc.tensor.matmul(ps[:, 1, :HB], bd, st[:, HB:],
                                     start=True, stop=True)
                    nc.gpsimd.tensor_mul(
                        out=stn[:, :J], in0=st[:, :J],
                        in1=ech[:, w0, col, :])
                    nc.vector.tensor_mul(
                        out=stn[:, J:HB], in0=ps[:, 0, J:HB],
                        in1=ech[:, w0 + 1:w0 + 8, col, :])
                    nc.vector.tensor_mul(
                        out=stn[:, HB:], in0=ps[:, 1, :HB],
                        in1=ech[:, w0 + 8:w0 + 16, col, :])
                elif g == 0:
                    nc.tensor.matmul(ps[:, 0, :HB], bd, st[:, :HB],
                                     start=True, stop=True)
                    nc.tensor.matmul(ps[:, 1, :HB], bd, st[:, HB:],
                                     start=True, stop=True)
                    nc.vector.tensor_mul(
                        out=stn, in0=ps[:, :, :HB],
                        in1=ech[:, w0:w0 + 16, col, :])
                elif g == NGRP - 1 and tau == NT - 1:
                    # freeze segment 31 (cols [SW-J:)): it keeps its state
                    nc.tensor.matmul(ps[:, 0, :HB], bd, st[:, :HB],
                                     start=True, stop=True)
                    nc.tensor.matmul(ps[:, 1, :HB - J], bd, st[:, HB:SW - J],
                                     start=True, stop=True)
                    nc.vector.tensor_mul(
                        out=stn[:, :HB], in0=ps[:, 0, :HB],
                        in1=ech[:, w0:w0 + 8, col, :])
                    nc.vector.tensor_mul(
                        out=stn[:, HB:SW - J], in0=ps[:, 1, :HB - J],
                        in1=ech[:, w0 + 8:w0 + 15, col, :])
                    nc.gpsimd.tensor_copy(out=stn[:, SW - J:],
                                          in_=st[:, SW - J:])
                else:
                    nc.tensor.matmul(ps[:, 0, :HB], bd, st[:, :HB],
                                     start=True, stop=True)
                    nc.tensor.matmul(ps[:, 1, :HB], bd, st[:, HB:],
                                     start=True, stop=True)
                    nc.vector.tensor_mul(
                        out=stn, in0=ps[:, :, :HB],
                        in1=ech[:, w0:w0 + 16, col, :])
                new_states.append(stn)
            states = new_states

            if tau == W - 1:
                ycs = colsums()
                nc.scalar.activation(
                    out=y_ln, in_=ycs[:, :, :HB],
                    func=mybir.ActivationFunctionType.Ln)
                # y telescope tree (Pool): exclude c=0
                nc.gpsimd.memset(y_ln[:, 0, :], 0.0)
                n = H
                while n > 1:
                    half = n // 2
                    nc.gpsimd.tensor_add(
                        out=y_ln[:, :half, :], in0=y_ln[:, :half, :],
                        in1=y_ln[:, half:n, :])
                    n = half
            elif tau >= GOLD_T0 and gold_thunks:
                gold_thunks.pop(0)()

        wcs = colsums()
        nc.tensor.matmul(wcs[:, 0, HB:HB + J], astop_bd,
                         states[NGRP - 1][:, SW - J:], start=True, stop=True)
        nc.tensor.matmul(wcs[:, 1, HB:HB + J], ones_f32, gacc,
                         start=True, stop=True)
        nc.scalar.activation(
            out=w_ln, in_=wcs[:, :, :HB],
            func=mybir.ActivationFunctionType.Ln)
        nll = small_pool.tile([G, J], F32, tag="nll")
        nc.scalar.activation(
            out=nll, in_=wcs[:, 0, HB:HB + J],
            func=mybir.ActivationFunctionType.Ln)

        nc.vector.memset(w_ln[:, H - 1, :], 0.0)   # w: exclude c=H-1
        n = H
        while n > 1:
            half = n // 2
            nc.vector.tensor_add(
                out=w_ln[:, :half, :], in0=w_ln[:, :half, :],
                in1=w_ln[:, half:n, :])
            n = half
        nc.vector.tensor_add(out=nll, in0=nll, in1=w_ln[:, 0, :])
        nc.vector.tensor_sub(out=nll, in0=nll, in1=y_ln[:, 0, :])
        nc.vector.tensor_sub(out=nll, in0=nll, in1=wcs[:, 1, HB:HB + J])
        nc.vector.tensor_scalar_add(out=nll, in0=nll, scalar1=C0 * float(T))
        nc.sync.dma_start(out=out_ext[:], in_=nll)

    nc.finalize()
    return nc


def _host_prep(feats, tags, transitions):
    """Per-core input maps. Pure layout/gather/dtype staging; the only host
    arithmetic is O(K^2) on the 11x11 transition matrix."""
    import ml_dtypes
    f8 = ml_dtypes.float8_e4m3fn
    bf16 = ml_dtypes.bfloat16
    feats = np.asarray(feats, dtype=np.float32)
    tags = np.asarray(tags).astype(np.int64)
    trans = np.asarray(transitions, dtype=np.float32)

    A = np.exp(trans.astype(np.float64)).astype(np.float32)
    Ab = A[:KT, :KT]
    a_start = A[:KT, START].astype(np.float32)
    a_stop = A[STOP, :KT].astype(np.float32)
    eye = np.eye(G, dtype=np.float32)

    bd = np.zeros((P, P), dtype=bf16)
    bd[:PL, :PL] = np.kron(eye, Ab.T).astype(bf16)
    ones_bd = np.zeros((P, G), dtype=bf16)
    ones_bd[:PL] = np.kron(eye, np.ones((KT, 1), np.float32)).astype(bf16)
    astop_bd = np.zeros((P, G), dtype=bf16)
    astop_bd[:PL] = np.kron(eye, a_stop.reshape(KT, 1)).astype(bf16)

    init = np.ones((P, NGRP * SW), dtype=np.float32)
    init[:, :J] = 0.0
    for g in range(G):
        init[g * KT:(g + 1) * KT, :J] = a_start[:, None]
    init[PL:] = 0.0
    init = init.astype(bf16)

    nslots = G * J

    in_maps = []
    for c in range(NCORES):
        fb = feats[c * BL:(c + 1) * BL, :, :KT]
        tb = tags[c * BL:(c + 1) * BL]

        emis = np.zeros((nslots, H * L, KT), dtype=np.float32)
        emis[:BL, W - 1:W - 1 + T] = fb
        if W > 1:
            emis[:, :W - 1] = C0
        emis8 = emis.astype(f8)

        main = emis8.reshape(nslots, H, NCH, TC, KT)
        full = main.transpose(0, 2, 1, 3, 4).copy()
        e_part = full.reshape(G, J, NCH, NWIN, TC, KT)
        e_part = e_part.transpose(0, 5, 2, 3, 4, 1)      # [G,KT,NCH,NWIN,TC,J]
        e8 = np.zeros((P, NCH, CHUNK_ELEMS), dtype=f8)
        e8[:PL] = e_part.reshape(PL, NCH, CHUNK_ELEMS)
        # chunk 0 ships col-major [TC, NWIN, J]
        ch0 = e8[:, 0, :].reshape(P, NWIN, TC, J).transpose(0, 2, 1, 3)
        e8[:, 0, :] = np.ascontiguousarray(ch0).reshape(P, CHUNK_ELEMS)

        tt = np.concatenate(
            [np.full((BL, 1), START, dtype=np.int64), tb], axis=1)
        ts = np.concatenate(
            [tb, np.full((BL, 1), STOP, dtype=np.int64)], axis=1)
        trans_sel = trans[ts, tt].astype(f8)
        feat_sel = np.take_along_axis(
            fb, tb[:, :, None], axis=2)[:, :, 0].astype(f8)
        gvals = np.zeros((nslots, KT * GT), dtype=f8)
        gvals[:BL, :T + 1] = trans_sel
        gvals[:BL, T + 1:2 * T + 1] = feat_sel
        gpart = gvals.reshape(G, J, KT, GT).transpose(0, 2, 3, 1)
        g8 = np.zeros((P, NGC, J * GC), dtype=f8)
        g8[:PL] = gpart.reshape(PL, NGC, GC, J).transpose(
            0, 1, 3, 2).reshape(PL, NGC, J * GC)

        in_maps.append({
            "emis8": e8,
            "gold8": g8,
            "bd_lhst": bd,
            "ones_bd": ones_bd,
            "astop_bd": astop_bd,
            "init_st": init,
            "ones_f32": ones_bd.astype(np.float32),
        })
    return in_maps


LAST_EXEC_NS = None


def kernel(feats, tags, transitions):
    global LAST_EXEC_NS
    in_maps = _host_prep(feats, tags, transitions)
    nc = _build_nc()
    trace = os.environ.get("KERNEL_TRACE") == "1"
    res = None
    for attempt in range(3):
        try:
            res = run_bass_kernel_spmd(
                nc, in_maps, list(range(NCORES)), trace=trace)
            break
        except Exception:
            if attempt == 2:
                raise
            import time as _time
            import jax as _jax
            try:
                _jax.clear_caches()
            except Exception:
                pass
            try:
                _jax.clear_backends()
            except Exception:
                pass
            _time.sleep(5)
    LAST_EXEC_NS = res.exec_time_ns
    outs = []
    for c in range(NCORES):
        nll_parts = np.asarray(res.results[c]["nll"], dtype=np.float32)
        outs.append(nll_parts.reshape(-1)[:BL])
    return np.concatenate(outs).astype(np.float32)


if __name__ == "__main__":
    rng = np.random.default_rng(0)
    feats = rng.standard_normal((B, T, K), dtype=np.float32)
    tags = rng.integers(0, 9, size=(B, T), dtype=np.int64)
    trans = rng.random((K, K), dtype=np.float32)
    trans[START, :] = -10000.0
    trans[:, STOP] = -10000.0
    out = kernel(feats=feats, tags=tags, transitions=trans)
    print(out.shape, out[:4])


# revision 3
# speedup vs baseline: 1.0772x; 1.0772x over previous
"""CRF NLL loss kernel v5 for Trainium2 (8 NeuronCores, batch-parallel).

H=64 segments of L=32 steps, zero warmup: serial depth 32 ticks. Chain c
starts from ones (segment 0 from e_0*a_start, pre-multiplied on device
from a tiny e0 input); the telescope's y-terms are then the constant
(H-1)*ln 9. Products of 32 positive matrices are rank-1 to ~1e-10, and
the 1-step-from-ones boundary directions cost only ~1e-3 relative error
(validated against the jax reference on the full batch).

Per tick: 2 groups x (4 PE matmuls [126,296] into 4 psum banks -> 1 DVE
tensor_mul [126,1184] via a strided 4-bank AP). All 8 psum banks carry
chain state; the final column-sum matmuls reuse those banks after the
last tick. Gold is off-chain (host-gathered values, Pool add-tree woven
into the later ticks).
"""
import os
import sys

import numpy as np

sys.path.insert(0, "/opt/trn_rl_repo")

from contextlib import ExitStack

import concourse.bacc as bacc
import concourse.bass as bass
import concourse.tile as tile
from concourse import mybir
from concourse.bass_utils import run_bass_kernel_spmd

# problem constants (hardcoded per spec)
B, T, K = 4096, 2048, 11
START, STOP = 10, 9
NCORES = 8
BL = B // NCORES          # 512
G, KT, J = 14, 9, 37
P = 128
PL = G * KT
H = 64                    # time segments
L = T // H                # 32
NT = L                    # 32 ticks, no warmup
TC = 2                    # ticks per emission chunk
NCH = L // TC             # 16 chunks
NWIN = H                  # 64 windows per chunk
NGRP = 2
SPG = H // NGRP           # 32 segments per group
SW = SPG * J              # 1184 state columns per group
NB = 4                    # psum banks per group
WB = SPG // NB            # 8 windows per bank
HB = WB * J               # 296 columns per bank
C0 = 3.25                 # fp8(e4m3)-exact recentering constant
LN9 = float(np.log(9.0))
GT = 512
NGC = 4
GC = GT // NGC            # 128
GOLD_T0 = 16

F32 = mybir.dt.float32
BF16 = mybir.dt.bfloat16
F8 = mybir.dt.float8e4

CHUNK_ELEMS = NWIN * TC * J


def _build_nc():
    nc = bacc.Bacc()
    e_in = nc.declare_dram_parameter(
        "emis8", [P, NCH, CHUNK_ELEMS], F8, isOutput=False)
    e0_in = nc.declare_dram_parameter("e0raw", [P, J], F8, isOutput=False)
    gold_in = nc.declare_dram_parameter(
        "gold8", [P, NGC, J * GC], F8, isOutput=False)
    bd_in = nc.declare_dram_parameter("bd_lhst", [P, P], BF16, isOutput=False)
    ones_in = nc.declare_dram_parameter("ones_bd", [P, G], BF16, isOutput=False)
    astop_in = nc.declare_dram_parameter("astop_bd", [P, G], BF16,
                                         isOutput=False)
    init_in = nc.declare_dram_parameter("init_st", [P, J], BF16,
                                        isOutput=False)
    onesf_in = nc.declare_dram_parameter("ones_f32", [P, G], F32,
                                         isOutput=False)
    out_ext = nc.declare_dram_parameter("nll", [G, J], F32, isOutput=True)

    with tile.TileContext(nc) as tc, ExitStack() as ctx:
        consts = ctx.enter_context(tc.tile_pool(name="consts", bufs=1))
        epool = ctx.enter_context(tc.tile_pool(name="epool", bufs=1))
        raw_pool = ctx.enter_context(tc.tile_pool(name="raw", bufs=2))
        state_pool = ctx.enter_context(tc.tile_pool(name="state", bufs=3))
        small_pool = ctx.enter_context(tc.tile_pool(name="small", bufs=3))
        psum_pool = ctx.enter_context(
            tc.tile_pool(name="psum", bufs=1, space="PSUM"))

        bias_c0 = consts.tile([P, 1], F32)
        nc.vector.memset(bias_c0, -C0)
        warm = consts.tile([P, 1], F32)
        nc.scalar.activation(
            out=warm, in_=bias_c0, func=mybir.ActivationFunctionType.Exp,
            bias=0.0, scale=1.0)
        # e0 for segment-0 init
        e0raw = consts.tile([P, J], F8)
        nc.sync.dma_start(out=e0raw, in_=e0_in[:])
        e0 = consts.tile([P, J], BF16)
        nc.scalar.activation(
            out=e0, in_=e0raw, func=mybir.ActivationFunctionType.Exp,
            bias=bias_c0, scale=1.0)

        # chunk 0 ships col-major [TC, NWIN, J]; DMA per col, exp per half
        echunks = [None] * NCH
        raw0 = raw_pool.tile([P, TC, NWIN, J], F8, tag="raw0")
        ech0 = epool.tile([P, NWIN, TC, J], F8, tag="ech0")
        nwj = NWIN * J
        for c in range(TC):
            nc.sync.dma_start(out=raw0[:, c],
                              in_=e_in[:, 0, c * nwj:(c + 1) * nwj])
            for hlf in range(2):
                s = hlf * SPG
                nc.scalar.activation(
                    out=ech0[:, s:s + SPG, c, :], in_=raw0[:, c, s:s + SPG],
                    func=mybir.ActivationFunctionType.Exp,
                    bias=bias_c0, scale=1.0)
        echunks[0] = ech0

        bd = consts.tile([P, P], BF16)
        nc.sync.dma_start(out=bd, in_=bd_in[:])
        init_st = consts.tile([P, J], BF16)
        nc.sync.dma_start(out=init_st, in_=init_in[:])
        ones_bd = consts.tile([P, G], BF16)
        nc.sync.dma_start(out=ones_bd, in_=ones_in[:])
        astop_bd = consts.tile([P, G], BF16)
        nc.sync.dma_start(out=astop_bd, in_=astop_in[:])
        ones_f32 = consts.tile([P, G], F32)
        nc.sync.dma_start(out=ones_f32, in_=onesf_in[:])

        states = []
        for g in range(NGRP):
            st = state_pool.tile([P, SW], BF16, tag=f"st{g}")
            nc.gpsimd.memset(st, 1.0)
            if g == 0:
                # segment 0: x0 = e_0 * a_start
                nc.gpsimd.tensor_mul(out=st[:, :J], in0=init_st, in1=e0)
            states.append(st)

        for k in range(1, NCH):
            raw = raw_pool.tile([P, NWIN, TC, J], F8, tag="raw")
            nc.sync.dma_start(out=raw, in_=e_in[:, k, :])
            ech = epool.tile([P, NWIN, TC, J], F8, tag=f"ech{k}")
            nc.scalar.activation(
                out=ech, in_=raw, func=mybir.ActivationFunctionType.Exp,
                bias=bias_c0, scale=1.0)
            echunks[k] = ech

        # gold: Pool add-tree woven into later ticks (<=2 thunks/tick)
        gacc = consts.tile([P, J], F32)
        nc.vector.memset(gacc, 0.0)
        glvl_a = consts.tile([P, J, GC // 2], F32)
        glvl_b = consts.tile([P, J, GC // 4], F32)
        glvl = [glvl_a, glvl_b]
        gold_thunks = []

        def make_gold_chunk(k):
            def dma():
                graw = raw_pool.tile([P, J, GC], F8, tag="graw")
                nc.sync.dma_start(out=graw, in_=gold_in[:, k, :])
                make_gold_chunk.cur = graw
            gold_thunks.append(dma)
            state = {"n": GC, "li": 0}

            def level(state=state):
                src = make_gold_chunk.cur
                n, li = state["n"], state["li"]
                half = n // 2
                if n > 1:
                    dst = glvl[li % 2]
                    nc.gpsimd.tensor_add(
                        out=dst[:, :, :half], in0=src[:, :, :half],
                        in1=src[:, :, half:n])
                    make_gold_chunk.cur = dst
                    state["n"], state["li"] = half, li + 1
                    if half == 1:
                        nc.gpsimd.tensor_add(
                            out=gacc, in0=gacc,
                            in1=make_gold_chunk.cur[:, :, 0])
            for _ in range(7):
                gold_thunks.append(level)
        for k in range(NGC):
            make_gold_chunk(k)

        w_ln = consts.tile([G, H, J], F32)

        for tau in range(NT):
            ech, col = echunks[tau // TC], tau % TC
            new_states = []
            for g in range(NGRP):
                st = states[g]
                stn = state_pool.tile([P, SW], BF16, tag=f"st{g}")
                ps = psum_pool.tile([P, NB, 512], F32, tag=f"ps{g}")
                w0 = SPG * g
                if g == NGRP - 1 and tau == NT - 1:
                    # freeze segment 63 (cols [SW-J:))
                    for b in range(NB - 1):
                        nc.tensor.matmul(
                            ps[:, b, :HB], bd, st[:, b * HB:(b + 1) * HB],
                            start=True, stop=True)
                    nc.tensor.matmul(
                        ps[:, NB - 1, :HB - J], bd,
                        st[:, (NB - 1) * HB:SW - J], start=True, stop=True)
                    nc.vector.tensor_mul(
                        out=stn[:, :(NB - 1) * HB], in0=ps[:, :NB - 1, :HB],
                        in1=ech[:, w0:w0 + (NB - 1) * WB, col, :])
                    nc.vector.tensor_mul(
                        out=stn[:, (NB - 1) * HB:SW - J],
                        in0=ps[:, NB - 1, :HB - J],
                        in1=ech[:, w0 + (NB - 1) * WB:w0 + SPG - 1, col, :])
                    nc.gpsimd.tensor_copy(out=stn[:, SW - J:],
                                          in_=st[:, SW - J:])
                else:
                    for b in range(NB):
                        nc.tensor.matmul(
                            ps[:, b, :HB], bd, st[:, b * HB:(b + 1) * HB],
                            start=True, stop=True)
                    nc.vector.tensor_mul(
                        out=stn, in0=ps[:, :, :HB],
                        in1=ech[:, w0:w0 + SPG, col, :])
                new_states.append(stn)
            states = new_states

            if tau >= GOLD_T0:
                for _ in range(2):
                    if gold_thunks:
                        gold_thunks.pop(0)()

        # w column-sums reuse the chain psum banks
        wcs = []
        for g in range(NGRP):
            cs = psum_pool.tile([G, NB, 512], F32, tag=f"ps{g}")
            for b in range(NB):
                nc.tensor.matmul(
                    cs[:, b, :HB], ones_bd,
                    states[g][:, b * HB:(b + 1) * HB], start=True, stop=True)
            wcs.append(cs)
        nc.tensor.matmul(wcs[0][:, 0, HB:HB + J], astop_bd,
                         states[NGRP - 1][:, SW - J:], start=True, stop=True)
        nc.tensor.matmul(wcs[0][:, 1, HB:HB + J], ones_f32, gacc,
                         start=True, stop=True)
        for g in range(NGRP):
            nc.scalar.activation(
                out=w_ln[:, g * SPG:(g + 1) * SPG, :], in_=wcs[g][:, :, :HB],
                func=mybir.ActivationFunctionType.Ln)
        nll = small_pool.tile([G, J], F32, tag="nll")
        nc.scalar.activation(
            out=nll, in_=wcs[0][:, 0, HB:HB + J],
            func=mybir.ActivationFunctionType.Ln)

        nc.vector.memset(w_ln[:, H - 1, :], 0.0)   # exclude c=H-1
        n = H
        while n > 1:
            half = n // 2
            nc.vector.tensor_add(
                out=w_ln[:, :half, :], in0=w_ln[:, :half, :],
                in1=w_ln[:, half:n, :])
            n = half
        nc.vector.tensor_add(out=nll, in0=nll, in1=w_ln[:, 0, :])
        nc.vector.tensor_sub(out=nll, in0=nll, in1=wcs[0][:, 1, HB:HB + J])
        nc.vector.tensor_scalar_add(
            out=nll, in0=nll, scalar1=C0 * float(T) - (H - 1) * LN9)
        nc.sync.dma_start(out=out_ext[:], in_=nll)

    nc.finalize()
    return nc


def _host_prep(feats, tags, transitions):
    """Per-core input maps. Pure layout/gather/dtype staging; the only host
    arithmetic is O(K^2) on the 11x11 transition matrix."""
    import ml_dtypes
    f8 = ml_dtypes.float8_e4m3fn
    bf16 = ml_dtypes.bfloat16
    feats = np.asarray(feats, dtype=np.float32)
    tags = np.asarray(tags).astype(np.int64)
    trans = np.asarray(transitions, dtype=np.float32)

    A = np.exp(trans.astype(np.float64)).astype(np.float32)
    Ab = A[:KT, :KT]
    a_start = A[:KT, START].astype(np.float32)
    a_stop = A[STOP, :KT].astype(np.float32)
    eye = np.eye(G, dtype=np.float32)

    bd = np.zeros((P, P), dtype=bf16)
    bd[:PL, :PL] = np.kron(eye, Ab.T).astype(bf16)
    ones_bd = np.zeros((P, G), dtype=bf16)
    ones_bd[:PL] = np.kron(eye, np.ones((KT, 1), np.float32)).astype(bf16)
    astop_bd = np.zeros((P, G), dtype=bf16)
    astop_bd[:PL] = np.kron(eye, a_stop.reshape(KT, 1)).astype(bf16)

    init = np.zeros((P, J), dtype=np.float32)
    for g in range(G):
        init[g * KT:(g + 1) * KT] = a_start[:, None]
    init = init.astype(bf16)

    nslots = G * J

    in_maps = []
    for c in range(NCORES):
        fb = feats[c * BL:(c + 1) * BL, :, :KT]
        tb = tags[c * BL:(c + 1) * BL]

        # emission tile: tilepos = t-1 for t in [1, T); last pos unused
        emis = np.zeros((nslots, H * L, KT), dtype=np.float32)
        emis[:BL, :T - 1] = fb[:, 1:]
        emis8 = emis.astype(f8)

        main = emis8.reshape(nslots, H, NCH, TC, KT)
        full = main.transpose(0, 2, 1, 3, 4).copy()     # [ns,NCH,NWIN,TC,KT]
        e_part = full.reshape(G, J, NCH, NWIN, TC, KT)
        e_part = e_part.transpose(0, 5, 2, 3, 4, 1)
        e8 = np.zeros((P, NCH, CHUNK_ELEMS), dtype=f8)
        e8[:PL] = e_part.reshape(PL, NCH, CHUNK_ELEMS)
        ch0 = e8[:, 0, :].reshape(P, NWIN, TC, J).transpose(0, 2, 1, 3)
        e8[:, 0, :] = np.ascontiguousarray(ch0).reshape(P, CHUNK_ELEMS)

        # e0: feat_0 per sentence, on its group's 9 tag-partitions
        e0 = np.zeros((nslots, KT), dtype=np.float32)
        e0[:BL] = fb[:, 0]
        e0p = np.zeros((P, J), dtype=f8)
        e0p[:PL] = e0.reshape(G, J, KT).transpose(0, 2, 1).reshape(PL, J)

        tt = np.concatenate(
            [np.full((BL, 1), START, dtype=np.int64), tb], axis=1)
        ts = np.concatenate(
            [tb, np.full((BL, 1), STOP, dtype=np.int64)], axis=1)
        trans_sel = trans[ts, tt].astype(f8)
        feat_sel = np.take_along_axis(
            fb, tb[:, :, None], axis=2)[:, :, 0].astype(f8)
        gvals = np.zeros((nslots, KT * GT), dtype=f8)
        gvals[:BL, :T + 1] = trans_sel
        gvals[:BL, T + 1:2 * T + 1] = feat_sel
        gpart = gvals.reshape(G, J, KT, GT).transpose(0, 2, 3, 1)
        g8 = np.zeros((P, NGC, J * GC), dtype=f8)
        g8[:PL] = gpart.reshape(PL, NGC, GC, J).transpose(
            0, 1, 3, 2).reshape(PL, NGC, J * GC)

        in_maps.append({
            "emis8": e8,
            "e0raw": e0p,
            "gold8": g8,
            "bd_lhst": bd,
            "ones_bd": ones_bd,
            "astop_bd": astop_bd,
            "init_st": init,
            "ones_f32": ones_bd.astype(np.float32),
        })
    return in_maps


LAST_EXEC_NS = None


def kernel(feats, tags, transitions):
    global LAST_EXEC_NS
    in_maps = _host_prep(feats, tags, transitions)
    nc = _build_nc()
    trace = os.environ.get("KERNEL_TRACE") == "1"
    res = None
    for attempt in range(3):
        try:
            res = run_bass_kernel_spmd(
                nc, in_maps, list(range(NCORES)), trace=trace)
            break
        except Exception:
            if attempt == 2:
                raise
            import time as _time
            import jax as _jax
            try:
                _jax.clear_caches()
            except Exception:
                pass
            try:
                _jax.clear_backends()
            except Exception:
                pass
            _time.sleep(5)
    LAST_EXEC_NS = res.exec_time_ns
    outs = []
    for c in range(NCORES):
        nll_parts = np.asarray(res.results[c]["nll"], dtype=np.float32)
        outs.append(nll_parts.reshape(-1)[:BL])
    return np.concatenate(outs).astype(np.float32)


if __name__ == "__main__":
    rng = np.random.default_rng(0)
    feats = rng.standard_normal((B, T, K), dtype=np.float32)
    tags = rng.integers(0, 9, size=(B, T), dtype=np.int64)
    trans = rng.random((K, K), dtype=np.float32)
    trans[START, :] = -10000.0
    trans[:, STOP] = -10000.0
    out = kernel(feats=feats, tags=tags, transitions=trans)
    print(out.shape, out[:4])


# revision 5
# speedup vs baseline: 1.1072x; 1.0279x over previous
"""CRF NLL loss kernel v5 for Trainium2 (8 NeuronCores, batch-parallel).

H=64 segments of L=32 steps, zero warmup: serial depth 32 ticks. Chain c
starts from ones (segment 0 from e_0*a_start, pre-multiplied on device
from a tiny e0 input); the telescope's y-terms are then the constant
(H-1)*ln 9. Products of 32 positive matrices are rank-1 to ~1e-10, and
the 1-step-from-ones boundary directions cost only ~1e-3 relative error
(validated against the jax reference on the full batch).

Per tick: 2 groups x (4 PE matmuls [126,296] into 4 psum banks -> 1 DVE
tensor_mul [126,1184] via a strided 4-bank AP). All 8 psum banks carry
chain state; the final column-sum matmuls reuse those banks after the
last tick. Gold is off-chain (host-gathered values, Pool add-tree woven
into the later ticks).
"""
import os
import sys

import numpy as np

sys.path.insert(0, "/opt/trn_rl_repo")

from contextlib import ExitStack

import concourse.bacc as bacc
import concourse.bass as bass
import concourse.tile as tile
from concourse import mybir
from concourse.bass_utils import run_bass_kernel_spmd

# problem constants (hardcoded per spec)
B, T, K = 4096, 2048, 11
START, STOP = 10, 9
NCORES = 8
BL = B // NCORES          # 512
G, KT, J = 14, 9, 37
P = 128
PL = G * KT
H = 64                    # time segments
L = T // H                # 32
NT = L                    # 32 ticks, no warmup
TC = 2                    # ticks per emission chunk
NCH = L // TC             # 16 chunks
NWIN = H                  # 64 windows per chunk
NGRP = 2
SPG = H // NGRP           # 32 segments per group
SW = SPG * J              # 1184 state columns per group
NB = 4                    # psum banks per group
WB = SPG // NB            # 8 windows per bank
HB = WB * J               # 296 columns per bank
C0 = 3.25                 # fp8(e4m3)-exact recentering constant
LN9 = float(np.log(9.0))
GT = 512
NGC = 4
GC = GT // NGC            # 128
GOLD_T0 = 16

F32 = mybir.dt.float32
BF16 = mybir.dt.bfloat16
F8 = mybir.dt.float8e4

CHUNK_ELEMS = NWIN * TC * J


def _build_nc():
    nc = bacc.Bacc()
    e_in = nc.declare_dram_parameter(
        "emis8", [P, NCH, CHUNK_ELEMS], F8, isOutput=False)
    e0_in = nc.declare_dram_parameter("e0raw", [P, J], F8, isOutput=False)
    gold_in = nc.declare_dram_parameter(
        "gold8", [P, NGC, J * GC], F8, isOutput=False)
    bd_in = nc.declare_dram_parameter("bd_lhst", [P, P], BF16, isOutput=False)
    ones_in = nc.declare_dram_parameter("ones_bd", [P, G], BF16, isOutput=False)
    astop_in = nc.declare_dram_parameter("astop_bd", [P, G], BF16,
                                         isOutput=False)
    init_in = nc.declare_dram_parameter("init_st", [P, J], BF16,
                                        isOutput=False)
    onesf_in = nc.declare_dram_parameter("ones_f32", [P, G], F32,
                                         isOutput=False)
    out_ext = nc.declare_dram_parameter("nll", [G, J], F32, isOutput=True)

    with tile.TileContext(nc) as tc, ExitStack() as ctx:
        consts = ctx.enter_context(tc.tile_pool(name="consts", bufs=1))
        epool = ctx.enter_context(tc.tile_pool(name="epool", bufs=1))
        raw_pool = ctx.enter_context(tc.tile_pool(name="raw", bufs=2))
        state_pool = ctx.enter_context(tc.tile_pool(name="state", bufs=3))
        small_pool = ctx.enter_context(tc.tile_pool(name="small", bufs=3))
        psum_pool = ctx.enter_context(
            tc.tile_pool(name="psum", bufs=1, space="PSUM"))

        bias_c0 = consts.tile([P, 1], F32)
        nc.vector.memset(bias_c0, -C0)
        warm = consts.tile([P, 1], F32)
        nc.scalar.activation(
            out=warm, in_=bias_c0, func=mybir.ActivationFunctionType.Exp,
            bias=0.0, scale=1.0)

        # chunk 0 ships col-major [TC, NWIN, J]; DMA per col, exp per half.
        # col-0 DMA is issued first; e0's exp rides between the col-0 halves.
        echunks = [None] * NCH
        raw0 = raw_pool.tile([P, TC, NWIN, J], F8, tag="raw0")
        ech0 = epool.tile([P, NWIN, TC, J], F8, tag="ech0")
        nwj = NWIN * J
        nc.sync.dma_start(out=raw0[:, 0], in_=e_in[:, 0, :nwj])
        e0raw = consts.tile([P, J], F8)
        nc.sync.dma_start(out=e0raw, in_=e0_in[:])
        e0 = consts.tile([P, J], BF16)
        nc.scalar.activation(
            out=ech0[:, :SPG, 0, :], in_=raw0[:, 0, :SPG],
            func=mybir.ActivationFunctionType.Exp, bias=bias_c0, scale=1.0)
        nc.scalar.activation(
            out=e0, in_=e0raw, func=mybir.ActivationFunctionType.Exp,
            bias=bias_c0, scale=1.0)
        nc.scalar.activation(
            out=ech0[:, SPG:, 0, :], in_=raw0[:, 0, SPG:],
            func=mybir.ActivationFunctionType.Exp, bias=bias_c0, scale=1.0)
        init_st = consts.tile([P, J], BF16)
        nc.sync.dma_start(out=init_st, in_=init_in[:])
        bd = consts.tile([P, P], BF16)
        nc.sync.dma_start(out=bd, in_=bd_in[:])
        nc.sync.dma_start(out=raw0[:, 1], in_=e_in[:, 0, nwj:2 * nwj])
        for hlf in range(2):
            s = hlf * SPG
            nc.scalar.activation(
                out=ech0[:, s:s + SPG, 1, :], in_=raw0[:, 1, s:s + SPG],
                func=mybir.ActivationFunctionType.Exp,
                bias=bias_c0, scale=1.0)
        echunks[0] = ech0

        ones_bd = consts.tile([P, G], BF16)
        nc.sync.dma_start(out=ones_bd, in_=ones_in[:])
        astop_bd = consts.tile([P, G], BF16)
        nc.sync.dma_start(out=astop_bd, in_=astop_in[:])
        ones_f32 = consts.tile([P, G], F32)
        nc.sync.dma_start(out=ones_f32, in_=onesf_in[:])

        states = []
        for g in range(NGRP):
            st = state_pool.tile([P, SW], BF16, tag=f"st{g}")
            nc.gpsimd.memset(st, 1.0)
            if g == 0:
                # segment 0: x0 = e_0 * a_start
                nc.gpsimd.tensor_mul(out=st[:, :J], in0=init_st, in1=e0)
            states.append(st)

        for k in range(1, NCH):
            raw = raw_pool.tile([P, NWIN, TC, J], F8, tag="raw")
            nc.sync.dma_start(out=raw, in_=e_in[:, k, :])
            ech = epool.tile([P, NWIN, TC, J], F8, tag=f"ech{k}")
            if k <= 3:
                # early chunks: exp per (col, group-half) so ticks 2..7
                # are not paced by whole-chunk 4us exps
                for c in range(TC):
                    for hlf in range(2):
                        s = hlf * SPG
                        nc.scalar.activation(
                            out=ech[:, s:s + SPG, c, :],
                            in_=raw[:, s:s + SPG, c, :],
                            func=mybir.ActivationFunctionType.Exp,
                            bias=bias_c0, scale=1.0)
            else:
                nc.scalar.activation(
                    out=ech, in_=raw, func=mybir.ActivationFunctionType.Exp,
                    bias=bias_c0, scale=1.0)
            echunks[k] = ech

        # gold: Pool add-tree woven into later ticks (<=2 thunks/tick)
        gacc = consts.tile([P, J], F32)
        nc.vector.memset(gacc, 0.0)
        glvl_a = consts.tile([P, J, GC // 2], F32)
        glvl_b = consts.tile([P, J, GC // 4], F32)
        glvl = [glvl_a, glvl_b]
        gold_thunks = []

        def make_gold_chunk(k):
            def dma():
                graw = raw_pool.tile([P, J, GC], F8, tag="graw")
                nc.sync.dma_start(out=graw, in_=gold_in[:, k, :])
                make_gold_chunk.cur = graw
            gold_thunks.append(dma)
            state = {"n": GC, "li": 0}

            def level(state=state):
                src = make_gold_chunk.cur
                n, li = state["n"], state["li"]
                half = n // 2
                if n > 1:
                    dst = glvl[li % 2]
                    nc.gpsimd.tensor_add(
                        out=dst[:, :, :half], in0=src[:, :, :half],
                        in1=src[:, :, half:n])
                    make_gold_chunk.cur = dst
                    state["n"], state["li"] = half, li + 1
                    if half == 1:
                        nc.gpsimd.tensor_add(
                            out=gacc, in0=gacc,
                            in1=make_gold_chunk.cur[:, :, 0])
            for _ in range(7):
                gold_thunks.append(level)
        for k in range(NGC):
            make_gold_chunk(k)

        w_ln = consts.tile([G, H, J], F32)

        w63_src = None
        for tau in range(NT):
            ech, col = echunks[tau // TC], tau % TC
            if tau == NT - 1:
                w63_src = states[1]   # seg-63's final state (tick 31 junks it)
            new_states = []
            for g in range(NGRP):
                st = states[g]
                stn = state_pool.tile([P, SW], BF16, tag=f"st{g}")
                ps = psum_pool.tile([P, NB, 512], F32, tag=f"ps{g}")
                w0 = SPG * g
                for b in range(NB):
                    nc.tensor.matmul(
                        ps[:, b, :HB], bd, st[:, b * HB:(b + 1) * HB],
                        start=True, stop=True)
                nc.vector.tensor_mul(
                    out=stn, in0=ps[:, :, :HB],
                    in1=ech[:, w0:w0 + SPG, col, :])
                new_states.append(stn)
            states = new_states

            if tau >= GOLD_T0:
                for _ in range(2):
                    if gold_thunks:
                        gold_thunks.pop(0)()

        # tail: per-bank column-sums/Lns so group 0's tree overlaps
        # group 1's Lns; the astop dot reads the saved tick-30 state.
        cs0 = psum_pool.tile([G, NB, 512], F32, tag="ps0")
        for b in range(NB):
            nc.tensor.matmul(cs0[:, b, :HB], ones_bd,
                             states[0][:, b * HB:(b + 1) * HB],
                             start=True, stop=True)
        for b in range(NB):
            nc.scalar.activation(
                out=w_ln[:, b * WB:(b + 1) * WB, :], in_=cs0[:, b, :HB],
                func=mybir.ActivationFunctionType.Ln)
        n = SPG
        while n > 1:
            half = n // 2
            nc.gpsimd.tensor_add(
                out=w_ln[:, :half, :], in0=w_ln[:, :half, :],
                in1=w_ln[:, half:n, :])
            n = half

        cs1 = psum_pool.tile([G, NB, 512], F32, tag="ps1")
        for b in range(NB):
            nc.tensor.matmul(cs1[:, b, :HB], ones_bd,
                             states[1][:, b * HB:(b + 1) * HB],
                             start=True, stop=True)
        nc.tensor.matmul(cs1[:, 2, HB:HB + J], astop_bd,
                         w63_src[:, SW - J:], start=True, stop=True)
        nc.tensor.matmul(cs1[:, 3, HB:HB + J], ones_f32, gacc,
                         start=True, stop=True)
        for b in range(NB):
            nc.scalar.activation(
                out=w_ln[:, SPG + b * WB:SPG + (b + 1) * WB, :],
                in_=cs1[:, b, :HB],
                func=mybir.ActivationFunctionType.Ln)
        nll = small_pool.tile([G, J], F32, tag="nll")
        nc.scalar.activation(
            out=nll, in_=cs1[:, 2, HB:HB + J],
            func=mybir.ActivationFunctionType.Ln)

        # group-1 tree, staggered: bank pairs as their Lns land
        nc.gpsimd.tensor_add(
            out=w_ln[:, SPG:SPG + WB, :], in0=w_ln[:, SPG:SPG + WB, :],
            in1=w_ln[:, SPG + WB:SPG + 2 * WB, :])
        nc.gpsimd.memset(w_ln[:, H - 1, :], 0.0)   # exclude c=H-1
        nc.gpsimd.tensor_add(
            out=w_ln[:, SPG + 2 * WB:SPG + 3 * WB, :],
            in0=w_ln[:, SPG + 2 * WB:SPG + 3 * WB, :],
            in1=w_ln[:, SPG + 3 * WB:SPG + 4 * WB, :])
        nc.gpsimd.tensor_add(
            out=w_ln[:, SPG:SPG + WB, :], in0=w_ln[:, SPG:SPG + WB, :],
            in1=w_ln[:, SPG + 2 * WB:SPG + 3 * WB, :])
        n = WB
        while n > 1:
            half = n // 2
            nc.gpsimd.tensor_add(
                out=w_ln[:, SPG:SPG + half, :], in0=w_ln[:, SPG:SPG + half, :],
                in1=w_ln[:, SPG + half:SPG + n, :])
            n = half
        nc.vector.tensor_add(out=nll, in0=nll, in1=w_ln[:, 0, :])
        nc.vector.tensor_add(out=nll, in0=nll, in1=w_ln[:, SPG, :])
        nc.vector.tensor_sub(out=nll, in0=nll, in1=cs1[:, 3, HB:HB + J])
        nc.vector.tensor_scalar_add(
            out=nll, in0=nll, scalar1=C0 * float(T) - (H - 1) * LN9)
        nc.sync.dma_start(out=out_ext[:], in_=nll)

    nc.finalize()
    return nc


def _host_prep(feats, tags, transitions):
    """Per-core input maps. Pure layout/gather/dtype staging; the only host
    arithmetic is O(K^2) on the 11x11 transition matrix."""
    import ml_dtypes
    f8 = ml_dtypes.float8_e4m3fn
    bf16 = ml_dtypes.bfloat16
    feats = np.asarray(feats, dtype=np.float32)
    tags = np.asarray(tags).astype(np.int64)
    trans = np.asarray(transitions, dtype=np.float32)

    A = np.exp(trans.astype(np.float64)).astype(np.float32)
    Ab = A[:KT, :KT]
    a_start = A[:KT, START].astype(np.float32)
    a_stop = A[STOP, :KT].astype(np.float32)
    eye = np.eye(G, dtype=np.float32)

    bd = np.zeros((P, P), dtype=bf16)
    bd[:PL, :PL] = np.kron(eye, Ab.T).astype(bf16)
    ones_bd = np.zeros((P, G), dtype=bf16)
    ones_bd[:PL] = np.kron(eye, np.ones((KT, 1), np.float32)).astype(bf16)
    astop_bd = np.zeros((P, G), dtype=bf16)
    astop_bd[:PL] = np.kron(eye, a_stop.reshape(KT, 1)).astype(bf16)

    init = np.zeros((P, J), dtype=np.float32)
    for g in range(G):
        init[g * KT:(g + 1) * KT] = a_start[:, None]
    init = init.astype(bf16)

    nslots = G * J

    in_maps = []
    for c in range(NCORES):
        fb = feats[c * BL:(c + 1) * BL, :, :KT]
        tb = tags[c * BL:(c + 1) * BL]

        # emission tile: tilepos = t-1 for t in [1, T); last pos unused
        emis = np.zeros((nslots, H * L, KT), dtype=np.float32)
        emis[:BL, :T - 1] = fb[:, 1:]
        emis8 = emis.astype(f8)

        main = emis8.reshape(nslots, H, NCH, TC, KT)
        full = main.transpose(0, 2, 1, 3, 4).copy()     # [ns,NCH,NWIN,TC,KT]
        e_part = full.reshape(G, J, NCH, NWIN, TC, KT)
        e_part = e_part.transpose(0, 5, 2, 3, 4, 1)
        e8 = np.zeros((P, NCH, CHUNK_ELEMS), dtype=f8)
        e8[:PL] = e_part.reshape(PL, NCH, CHUNK_ELEMS)
        ch0 = e8[:, 0, :].reshape(P, NWIN, TC, J).transpose(0, 2, 1, 3)
        e8[:, 0, :] = np.ascontiguousarray(ch0).reshape(P, CHUNK_ELEMS)

        # e0: feat_0 per sentence, on its group's 9 tag-partitions
        e0 = np.zeros((nslots, KT), dtype=np.float32)
        e0[:BL] = fb[:, 0]
        e0p = np.zeros((P, J), dtype=f8)
        e0p[:PL] = e0.reshape(G, J, KT).transpose(0, 2, 1).reshape(PL, J)

        tt = np.concatenate(
            [np.full((BL, 1), START, dtype=np.int64), tb], axis=1)
        ts = np.concatenate(
            [tb, np.full((BL, 1), STOP, dtype=np.int64)], axis=1)
        trans_sel = trans[ts, tt].astype(f8)
        feat_sel = np.take_along_axis(
            fb, tb[:, :, None], axis=2)[:, :, 0].astype(f8)
        gvals = np.zeros((nslots, KT * GT), dtype=f8)
        gvals[:BL, :T + 1] = trans_sel
        gvals[:BL, T + 1:2 * T + 1] = feat_sel
        gpart = gvals.reshape(G, J, KT, GT).transpose(0, 2, 3, 1)
        g8 = np.zeros((P, NGC, J * GC), dtype=f8)
        g8[:PL] = gpart.reshape(PL, NGC, GC, J).transpose(
            0, 1, 3, 2).reshape(PL, NGC, J * GC)

        in_maps.append({
            "emis8": e8,
            "e0raw": e0p,
            "gold8": g8,
            "bd_lhst": bd,
            "ones_bd": ones_bd,
            "astop_bd": astop_bd,
            "init_st": init,
            "ones_f32": ones_bd.astype(np.float32),
        })
    return in_maps


LAST_EXEC_NS = None


def kernel(feats, tags, transitions):
    global LAST_EXEC_NS
    in_maps = _host_prep(feats, tags, transitions)
    nc = _build_nc()
    trace = os.environ.get("KERNEL_TRACE") == "1"
    res = None
    for attempt in range(3):
        try:
            res = run_bass_kernel_spmd(
                nc, in_maps, list(range(NCORES)), trace=trace)
            break
        except Exception:
            if attempt == 2:
                raise
            import time as _time
            import jax as _jax
            try:
                _jax.clear_caches()
            except Exception:
                pass
            try:
                _jax.clear_backends()
            except Exception:
                pass
            _time.sleep(5)
    LAST_EXEC_NS = res.exec_time_ns
    outs = []
    for c in range(NCORES):
        nll_parts = np.asarray(res.results[c]["nll"], dtype=np.float32)
        outs.append(nll_parts.reshape(-1)[:BL])
    return np.concatenate(outs).astype(np.float32)


if __name__ == "__main__":
    rng = np.random.default_rng(0)
    feats = rng.standard_normal((B, T, K), dtype=np.float32)
    tags = rng.integers(0, 9, size=(B, T), dtype=np.int64)
    trans = rng.random((K, K), dtype=np.float32)
    trans[START, :] = -10000.0
    trans[:, STOP] = -10000.0
    out = kernel(feats=feats, tags=tags, transitions=trans)
    print(out.shape, out[:4])


# revision 6
# speedup vs baseline: 1.1145x; 1.0066x over previous
"""CRF NLL loss kernel v5 for Trainium2 (8 NeuronCores, batch-parallel).

H=64 segments of L=32 steps, zero warmup: serial depth 32 ticks. Chain c
starts from ones (segment 0 from e_0*a_start, pre-multiplied on device
from a tiny e0 input); the telescope's y-terms are then the constant
(H-1)*ln 9. Products of 32 positive matrices are rank-1 to ~1e-10, and
the 1-step-from-ones boundary directions cost only ~1e-3 relative error
(validated against the jax reference on the full batch).

Per tick: 2 groups x (4 PE matmuls [126,296] into 4 psum banks -> 1 DVE
tensor_mul [126,1184] via a strided 4-bank AP). All 8 psum banks carry
chain state; the final column-sum matmuls reuse those banks after the
last tick. Gold is off-chain (host-gathered values, Pool add-tree woven
into the later ticks).
"""
import os
import sys

import numpy as np

sys.path.insert(0, "/opt/trn_rl_repo")

from contextlib import ExitStack

import concourse.bacc as bacc
import concourse.bass as bass
import concourse.tile as tile
from concourse import mybir
from concourse.bass_utils import run_bass_kernel_spmd

# problem constants (hardcoded per spec)
B, T, K = 4096, 2048, 11
START, STOP = 10, 9
NCORES = 8
BL = B // NCORES          # 512
G, KT, J = 14, 9, 37
P = 128
PL = G * KT
H = 64                    # time segments
L = T // H                # 32
NT = L                    # 32 ticks, no warmup
TC = 2                    # ticks per emission chunk
NCH = L // TC             # 16 chunks
NWIN = H                  # 64 windows per chunk
NGRP = 2
SPG = H // NGRP           # 32 segments per group
SW = SPG * J              # 1184 state columns per group
NB = 4                    # psum banks per group
WB = SPG // NB            # 8 windows per bank
HB = WB * J               # 296 columns per bank
C0 = 3.25                 # fp8(e4m3)-exact recentering constant
LN9 = float(np.log(9.0))
GT = 512
NGC = 4
GC = GT // NGC            # 128
GOLD_T0 = 16

F32 = mybir.dt.float32
BF16 = mybir.dt.bfloat16
F8 = mybir.dt.float8e4

CHUNK_ELEMS = NWIN * TC * J


def _build_nc():
    nc = bacc.Bacc()
    e_in = nc.declare_dram_parameter(
        "emis8", [P, NCH, CHUNK_ELEMS], F8, isOutput=False)
    e0_in = nc.declare_dram_parameter("e0raw", [P, J], F8, isOutput=False)
    gold_in = nc.declare_dram_parameter(
        "gold8", [P, NGC, J * GC], F8, isOutput=False)
    bd_in = nc.declare_dram_parameter("bd_lhst", [P, P], BF16, isOutput=False)
    ones_in = nc.declare_dram_parameter("ones_bd", [P, G], BF16, isOutput=False)
    astop_in = nc.declare_dram_parameter("astop_bd", [P, G], BF16,
                                         isOutput=False)
    init_in = nc.declare_dram_parameter("init_st", [P, J], BF16,
                                        isOutput=False)
    onesf_in = nc.declare_dram_parameter("ones_f32", [P, G], F32,
                                         isOutput=False)
    out_ext = nc.declare_dram_parameter("nll", [G, J], F32, isOutput=True)

    with tile.TileContext(nc) as tc, ExitStack() as ctx:
        consts = ctx.enter_context(tc.tile_pool(name="consts", bufs=1))
        epool = ctx.enter_context(tc.tile_pool(name="epool", bufs=1))
        raw_pool = ctx.enter_context(tc.tile_pool(name="raw", bufs=2))
        state_pool = ctx.enter_context(tc.tile_pool(name="state", bufs=3))
        small_pool = ctx.enter_context(tc.tile_pool(name="small", bufs=3))
        psum_pool = ctx.enter_context(
            tc.tile_pool(name="psum", bufs=1, space="PSUM"))

        bias_c0 = consts.tile([P, 1], F32)
        nc.vector.memset(bias_c0, -C0)
        warm = consts.tile([P, 1], F32)
        nc.scalar.activation(
            out=warm, in_=bias_c0, func=mybir.ActivationFunctionType.Exp,
            bias=0.0, scale=1.0)

        # chunk 0 ships col-major [TC, NWIN, J]; DMA per col, exp per half.
        # col-0 DMA is issued first; e0's exp rides between the col-0 halves.
        echunks = [None] * NCH
        raw0 = raw_pool.tile([P, TC, NWIN, J], F8, tag="raw0")
        ech0 = epool.tile([P, NWIN, TC, J], F8, tag="ech0")
        nwj = NWIN * J
        nc.sync.dma_start(out=raw0[:, 0], in_=e_in[:, 0, :nwj])
        e0raw = consts.tile([P, J], F8)
        nc.sync.dma_start(out=e0raw, in_=e0_in[:])
        e0 = consts.tile([P, J], BF16)
        nc.scalar.activation(
            out=ech0[:, :SPG, 0, :], in_=raw0[:, 0, :SPG],
            func=mybir.ActivationFunctionType.Exp, bias=bias_c0, scale=1.0)
        nc.scalar.activation(
            out=e0, in_=e0raw, func=mybir.ActivationFunctionType.Exp,
            bias=bias_c0, scale=1.0)
        nc.scalar.activation(
            out=ech0[:, SPG:, 0, :], in_=raw0[:, 0, SPG:],
            func=mybir.ActivationFunctionType.Exp, bias=bias_c0, scale=1.0)
        init_st = consts.tile([P, J], BF16)
        nc.sync.dma_start(out=init_st, in_=init_in[:])
        bd = consts.tile([P, P], BF16)
        nc.sync.dma_start(out=bd, in_=bd_in[:])
        nc.sync.dma_start(out=raw0[:, 1], in_=e_in[:, 0, nwj:2 * nwj])
        for hlf in range(2):
            s = hlf * SPG
            nc.scalar.activation(
                out=ech0[:, s:s + SPG, 1, :], in_=raw0[:, 1, s:s + SPG],
                func=mybir.ActivationFunctionType.Exp,
                bias=bias_c0, scale=1.0)
        echunks[0] = ech0

        ones_bd = consts.tile([P, G], BF16)
        nc.sync.dma_start(out=ones_bd, in_=ones_in[:])
        astop_bd = consts.tile([P, G], BF16)
        nc.sync.dma_start(out=astop_bd, in_=astop_in[:])
        ones_f32 = consts.tile([P, G], F32)
        nc.sync.dma_start(out=ones_f32, in_=onesf_in[:])

        states = []
        for g in range(NGRP):
            st = state_pool.tile([P, SW], BF16, tag=f"st{g}")
            nc.gpsimd.memset(st, 1.0)
            if g == 0:
                # segment 0: x0 = e_0 * a_start
                nc.gpsimd.tensor_mul(out=st[:, :J], in0=init_st, in1=e0)
            states.append(st)

        for k in range(1, NCH):
            raw = raw_pool.tile([P, NWIN, TC, J], F8, tag="raw")
            nc.sync.dma_start(out=raw, in_=e_in[:, k, :])
            ech = epool.tile([P, NWIN, TC, J], F8, tag=f"ech{k}")
            if k <= 3:
                # early chunks: exp per (col, group-half) so ticks 2..7
                # are not paced by whole-chunk 4us exps
                for c in range(TC):
                    for hlf in range(2):
                        s = hlf * SPG
                        nc.scalar.activation(
                            out=ech[:, s:s + SPG, c, :],
                            in_=raw[:, s:s + SPG, c, :],
                            func=mybir.ActivationFunctionType.Exp,
                            bias=bias_c0, scale=1.0)
            else:
                nc.scalar.activation(
                    out=ech, in_=raw, func=mybir.ActivationFunctionType.Exp,
                    bias=bias_c0, scale=1.0)
            echunks[k] = ech

        # gold: Pool add-tree woven into later ticks (<=2 thunks/tick)
        gacc = consts.tile([P, J], F32)
        nc.vector.memset(gacc, 0.0)
        glvl_a = consts.tile([P, J, GC // 2], F32)
        glvl_b = consts.tile([P, J, GC // 4], F32)
        glvl = [glvl_a, glvl_b]
        gold_thunks = []

        def make_gold_chunk(k):
            def dma():
                graw = raw_pool.tile([P, J, GC], F8, tag="graw")
                nc.sync.dma_start(out=graw, in_=gold_in[:, k, :])
                make_gold_chunk.cur = graw
            gold_thunks.append(dma)
            state = {"n": GC, "li": 0}

            def level(state=state):
                src = make_gold_chunk.cur
                n, li = state["n"], state["li"]
                half = n // 2
                if n > 1:
                    dst = glvl[li % 2]
                    nc.gpsimd.tensor_add(
                        out=dst[:, :, :half], in0=src[:, :, :half],
                        in1=src[:, :, half:n])
                    make_gold_chunk.cur = dst
                    state["n"], state["li"] = half, li + 1
                    if half == 1:
                        nc.gpsimd.tensor_add(
                            out=gacc, in0=gacc,
                            in1=make_gold_chunk.cur[:, :, 0])
            for _ in range(7):
                gold_thunks.append(level)
        for k in range(NGC):
            make_gold_chunk(k)

        w_ln = consts.tile([G, H, J], F32)

        w63_src = None
        for tau in range(NT):
            ech, col = echunks[tau // TC], tau % TC
            if tau == NT - 1:
                w63_src = states[1]   # seg-63's final state (tick 31 junks it)
            new_states = []
            for g in range(NGRP):
                st = states[g]
                stn = state_pool.tile([P, SW], BF16, tag=f"st{g}")
                ps = psum_pool.tile([P, NB, 512], F32, tag=f"ps{g}")
                w0 = SPG * g
                for b in range(NB):
                    nc.tensor.matmul(
                        ps[:, b, :HB], bd, st[:, b * HB:(b + 1) * HB],
                        start=True, stop=True)
                nc.vector.tensor_mul(
                    out=stn, in0=ps[:, :, :HB],
                    in1=ech[:, w0:w0 + SPG, col, :])
                new_states.append(stn)
            states = new_states

            if tau >= GOLD_T0:
                for _ in range(2):
                    if gold_thunks:
                        gold_thunks.pop(0)()

        # tail: per-bank column-sums/Lns so group 0's tree overlaps
        # group 1's Lns; the astop dot reads the saved tick-30 state.
        cs0 = psum_pool.tile([G, NB, 512], F32, tag="ps0")
        for b in range(NB):
            nc.tensor.matmul(cs0[:, b, :HB], ones_bd,
                             states[0][:, b * HB:(b + 1) * HB],
                             start=True, stop=True)
        nc.scalar.activation(
            out=w_ln[:, :SPG, :], in_=cs0[:, :, :HB],
            func=mybir.ActivationFunctionType.Ln)
        n = SPG
        while n > 1:
            half = n // 2
            nc.gpsimd.tensor_add(
                out=w_ln[:, :half, :], in0=w_ln[:, :half, :],
                in1=w_ln[:, half:n, :])
            n = half

        cs1 = psum_pool.tile([G, NB, 512], F32, tag="ps1")
        for b in range(NB):
            nc.tensor.matmul(cs1[:, b, :HB], ones_bd,
                             states[1][:, b * HB:(b + 1) * HB],
                             start=True, stop=True)
        nc.tensor.matmul(cs1[:, 2, HB:HB + J], astop_bd,
                         w63_src[:, SW - J:], start=True, stop=True)
        nc.tensor.matmul(cs1[:, 3, HB:HB + J], ones_f32, gacc,
                         start=True, stop=True)
        nc.scalar.activation(
            out=w_ln[:, SPG:, :], in_=cs1[:, :, :HB],
            func=mybir.ActivationFunctionType.Ln)
        nll = small_pool.tile([G, J], F32, tag="nll")
        nc.scalar.activation(
            out=nll, in_=cs1[:, 2, HB:HB + J],
            func=mybir.ActivationFunctionType.Ln)

        # group-1 tree, staggered: bank pairs as their Lns land
        nc.gpsimd.tensor_add(
            out=w_ln[:, SPG:SPG + WB, :], in0=w_ln[:, SPG:SPG + WB, :],
            in1=w_ln[:, SPG + WB:SPG + 2 * WB, :])
        nc.gpsimd.memset(w_ln[:, H - 1, :], 0.0)   # exclude c=H-1
        nc.gpsimd.tensor_add(
            out=w_ln[:, SPG + 2 * WB:SPG + 3 * WB, :],
            in0=w_ln[:, SPG + 2 * WB:SPG + 3 * WB, :],
            in1=w_ln[:, SPG + 3 * WB:SPG + 4 * WB, :])
        nc.gpsimd.tensor_add(
            out=w_ln[:, SPG:SPG + WB, :], in0=w_ln[:, SPG:SPG + WB, :],
            in1=w_ln[:, SPG + 2 * WB:SPG + 3 * WB, :])
        n = WB
        while n > 1:
            half = n // 2
            nc.gpsimd.tensor_add(
                out=w_ln[:, SPG:SPG + half, :], in0=w_ln[:, SPG:SPG + half, :],
                in1=w_ln[:, SPG + half:SPG + n, :])
            n = half
        nc.vector.tensor_add(out=nll, in0=nll, in1=w_ln[:, 0, :])
        nc.vector.tensor_add(out=nll, in0=nll, in1=w_ln[:, SPG, :])
        nc.vector.tensor_sub(out=nll, in0=nll, in1=cs1[:, 3, HB:HB + J])
        nc.vector.tensor_scalar_add(
            out=nll, in0=nll, scalar1=C0 * float(T) - (H - 1) * LN9)
        nc.sync.dma_start(out=out_ext[:], in_=nll)

    nc.finalize()
    return nc


def _host_prep(feats, tags, transitions):
    """Per-core input maps. Pure layout/gather/dtype staging; the only host
    arithmetic is O(K^2) on the 11x11 transition matrix."""
    import ml_dtypes
    f8 = ml_dtypes.float8_e4m3fn
    bf16 = ml_dtypes.bfloat16
    feats = np.asarray(feats, dtype=np.float32)
    tags = np.asarray(tags).astype(np.int64)
    trans = np.asarray(transitions, dtype=np.float32)

    A = np.exp(trans.astype(np.float64)).astype(np.float32)
    Ab = A[:KT, :KT]
    a_start = A[:KT, START].astype(np.float32)
    a_stop = A[STOP, :KT].astype(np.float32)
    eye = np.eye(G, dtype=np.float32)

    bd = np.zeros((P, P), dtype=bf16)
    bd[:PL, :PL] = np.kron(eye, Ab.T).astype(bf16)
    ones_bd = np.zeros((P, G), dtype=bf16)
    ones_bd[:PL] = np.kron(eye, np.ones((KT, 1), np.float32)).astype(bf16)
    astop_bd = np.zeros((P, G), dtype=bf16)
    astop_bd[:PL] = np.kron(eye, a_stop.reshape(KT, 1)).astype(bf16)

    init = np.zeros((P, J), dtype=np.float32)
    for g in range(G):
        init[g * KT:(g + 1) * KT] = a_start[:, None]
    init = init.astype(bf16)

    nslots = G * J

    in_maps = []
    for c in range(NCORES):
        fb = feats[c * BL:(c + 1) * BL, :, :KT]
        tb = tags[c * BL:(c + 1) * BL]

        # emission tile: tilepos = t-1 for t in [1, T); last pos unused
        emis = np.zeros((nslots, H * L, KT), dtype=np.float32)
        emis[:BL, :T - 1] = fb[:, 1:]
        emis8 = emis.astype(f8)

        main = emis8.reshape(nslots, H, NCH, TC, KT)
        full = main.transpose(0, 2, 1, 3, 4).copy()     # [ns,NCH,NWIN,TC,KT]
        e_part = full.reshape(G, J, NCH, NWIN, TC, KT)
        e_part = e_part.transpose(0, 5, 2, 3, 4, 1)
        e8 = np.zeros((P, NCH, CHUNK_ELEMS), dtype=f8)
        e8[:PL] = e_part.reshape(PL, NCH, CHUNK_ELEMS)
        ch0 = e8[:, 0, :].reshape(P, NWIN, TC, J).transpose(0, 2, 1, 3)
        e8[:, 0, :] = np.ascontiguousarray(ch0).reshape(P, CHUNK_ELEMS)

        # e0: feat_0 per sentence, on its group's 9 tag-partitions
        e0 = np.zeros((nslots, KT), dtype=np.float32)
        e0[:BL] = fb[:, 0]
        e0p = np.zeros((P, J), dtype=f8)
        e0p[:PL] = e0.reshape(G, J, KT).transpose(0, 2, 1).reshape(PL, J)

        tt = np.concatenate(
            [np.full((BL, 1), START, dtype=np.int64), tb], axis=1)
        ts = np.concatenate(
            [tb, np.full((BL, 1), STOP, dtype=np.int64)], axis=1)
        trans_sel = trans[ts, tt].astype(f8)
        feat_sel = np.take_along_axis(
            fb, tb[:, :, None], axis=2)[:, :, 0].astype(f8)
        gvals = np.zeros((nslots, KT * GT), dtype=f8)
        gvals[:BL, :T + 1] = trans_sel
        gvals[:BL, T + 1:2 * T + 1] = feat_sel
        gpart = gvals.reshape(G, J, KT, GT).transpose(0, 2, 3, 1)
        g8 = np.zeros((P, NGC, J * GC), dtype=f8)
        g8[:PL] = gpart.reshape(PL, NGC, GC, J).transpose(
            0, 1, 3, 2).reshape(PL, NGC, J * GC)

        in_maps.append({
            "emis8": e8,
            "e0raw": e0p,
            "gold8": g8,
            "bd_lhst": bd,
            "ones_bd": ones_bd,
            "astop_bd": astop_bd,
            "init_st": init,
            "ones_f32": ones_bd.astype(np.float32),
        })
    return in_maps


LAST_EXEC_NS = None


def kernel(feats, tags, transitions):
    global LAST_EXEC_NS
    in_maps = _host_prep(feats, tags, transitions)
    nc = _build_nc()
    trace = os.environ.get("KERNEL_TRACE") == "1"
    res = None
    for attempt in range(3):
        try:
            res = run_bass_kernel_spmd(
                nc, in_maps, list(range(NCORES)), trace=trace)
            break
        except Exception:
            if attempt == 2:
                raise
            import time as _time
            import jax as _jax
            try:
                _jax.clear_caches()
            except Exception:
                pass
            try:
                _jax.clear_backends()
            except Exception:
                pass
            _time.sleep(5)
    LAST_EXEC_NS = res.exec_time_ns
    outs = []
    for c in range(NCORES):
        nll_parts = np.asarray(res.results[c]["nll"], dtype=np.float32)
        outs.append(nll_parts.reshape(-1)[:BL])
    return np.concatenate(outs).astype(np.float32)


if __name__ == "__main__":
    rng = np.random.default_rng(0)
    feats = rng.standard_normal((B, T, K), dtype=np.float32)
    tags = rng.integers(0, 9, size=(B, T), dtype=np.int64)
    trans = rng.random((K, K), dtype=np.float32)
    trans[START, :] = -10000.0
    trans[:, STOP] = -10000.0
    out = kernel(feats=feats, tags=tags, transitions=trans)
    print(out.shape, out[:4])


# revision 7
# speedup vs baseline: 1.1173x; 1.0025x over previous
"""CRF NLL loss kernel v5 for Trainium2 (8 NeuronCores, batch-parallel).

H=64 segments of L=32 steps, zero warmup: serial depth 32 ticks. Chain c
starts from ones (segment 0 from e_0*a_start, pre-multiplied on device
from a tiny e0 input); the telescope's y-terms are then the constant
(H-1)*ln 9. Products of 32 positive matrices are rank-1 to ~1e-10, and
the 1-step-from-ones boundary directions cost only ~1e-3 relative error
(validated against the jax reference on the full batch).

Per tick: 2 groups x (4 PE matmuls [126,296] into 4 psum banks -> 1 DVE
tensor_mul [126,1184] via a strided 4-bank AP). All 8 psum banks carry
chain state; the final column-sum matmuls reuse those banks after the
last tick. Gold is off-chain (host-gathered values, Pool add-tree woven
into the later ticks).
"""
import os
import sys

import numpy as np

sys.path.insert(0, "/opt/trn_rl_repo")

from contextlib import ExitStack

import concourse.bacc as bacc
import concourse.bass as bass
import concourse.tile as tile
from concourse import mybir
from concourse.bass_utils import run_bass_kernel_spmd

# problem constants (hardcoded per spec)
B, T, K = 4096, 2048, 11
START, STOP = 10, 9
NCORES = 8
BL = B // NCORES          # 512
G, KT, J = 14, 9, 37
P = 128
PL = G * KT
H = 64                    # time segments
L = T // H                # 32
NT = L                    # 32 ticks, no warmup
TC = 2                    # ticks per emission chunk
NCH = L // TC             # 16 chunks
NWIN = H                  # 64 windows per chunk
NGRP = 2
SPG = H // NGRP           # 32 segments per group
SW = SPG * J              # 1184 state columns per group
NB = 4                    # psum banks per group
WB = SPG // NB            # 8 windows per bank
HB = WB * J               # 296 columns per bank
C0 = 3.25                 # fp8(e4m3)-exact recentering constant
LN9 = float(np.log(9.0))
GT = 512
NGC = 4
GC = GT // NGC            # 128
GOLD_T0 = 16

F32 = mybir.dt.float32
BF16 = mybir.dt.bfloat16
F8 = mybir.dt.float8e4

CHUNK_ELEMS = NWIN * TC * J


def _build_nc():
    nc = bacc.Bacc()
    e_in = nc.declare_dram_parameter(
        "emis8", [P, NCH, CHUNK_ELEMS], F8, isOutput=False)
    e0_in = nc.declare_dram_parameter("e0raw", [P, J], F8, isOutput=False)
    gold_in = nc.declare_dram_parameter(
        "gold8", [P, NGC, J * GC], F8, isOutput=False)
    bd_in = nc.declare_dram_parameter("bd_lhst", [P, P], BF16, isOutput=False)
    ones_in = nc.declare_dram_parameter("ones_bd", [P, G], BF16, isOutput=False)
    astop_in = nc.declare_dram_parameter("astop_bd", [P, G], BF16,
                                         isOutput=False)
    init_in = nc.declare_dram_parameter("init_st", [P, J], BF16,
                                        isOutput=False)
    onesf_in = nc.declare_dram_parameter("ones_f32", [P, G], F32,
                                         isOutput=False)
    out_ext = nc.declare_dram_parameter("nll", [G, J], F32, isOutput=True)

    with tile.TileContext(nc) as tc, ExitStack() as ctx:
        consts = ctx.enter_context(tc.tile_pool(name="consts", bufs=1))
        epool = ctx.enter_context(tc.tile_pool(name="epool", bufs=1))
        raw_pool = ctx.enter_context(tc.tile_pool(name="raw", bufs=2))
        state_pool = ctx.enter_context(tc.tile_pool(name="state", bufs=3))
        small_pool = ctx.enter_context(tc.tile_pool(name="small", bufs=3))
        psum_pool = ctx.enter_context(
            tc.tile_pool(name="psum", bufs=1, space="PSUM"))

        bias_c0 = consts.tile([P, 1], F32)
        nc.vector.memset(bias_c0, -C0)
        warm = consts.tile([P, 1], F32)
        nc.scalar.activation(
            out=warm, in_=bias_c0, func=mybir.ActivationFunctionType.Exp,
            bias=0.0, scale=1.0)

        # chunk 0 ships col-major [TC, NWIN, J]; DMA per col, exp per half.
        # col-0 DMA is issued first; e0's exp rides between the col-0 halves.
        echunks = [None] * NCH
        raw0 = raw_pool.tile([P, TC, NWIN, J], F8, tag="raw0")
        ech0 = epool.tile([P, NWIN, TC, J], F8, tag="ech0")
        nwj = NWIN * J
        e0raw = consts.tile([P, J], F8)
        nc.sync.dma_start(out=e0raw, in_=e0_in[:])
        nc.sync.dma_start(out=raw0[:, 0], in_=e_in[:, 0, :nwj])
        e0 = consts.tile([P, J], BF16)
        nc.scalar.activation(
            out=e0, in_=e0raw, func=mybir.ActivationFunctionType.Exp,
            bias=bias_c0, scale=1.0)
        nc.scalar.activation(
            out=ech0[:, :SPG, 0, :], in_=raw0[:, 0, :SPG],
            func=mybir.ActivationFunctionType.Exp, bias=bias_c0, scale=1.0)
        nc.scalar.activation(
            out=ech0[:, SPG:, 0, :], in_=raw0[:, 0, SPG:],
            func=mybir.ActivationFunctionType.Exp, bias=bias_c0, scale=1.0)
        init_st = consts.tile([P, J], BF16)
        nc.sync.dma_start(out=init_st, in_=init_in[:])
        bd = consts.tile([P, P], BF16)
        nc.sync.dma_start(out=bd, in_=bd_in[:])
        nc.sync.dma_start(out=raw0[:, 1], in_=e_in[:, 0, nwj:2 * nwj])
        for hlf in range(2):
            s = hlf * SPG
            nc.scalar.activation(
                out=ech0[:, s:s + SPG, 1, :], in_=raw0[:, 1, s:s + SPG],
                func=mybir.ActivationFunctionType.Exp,
                bias=bias_c0, scale=1.0)
        echunks[0] = ech0

        ones_bd = consts.tile([P, G], BF16)
        nc.sync.dma_start(out=ones_bd, in_=ones_in[:])
        astop_bd = consts.tile([P, G], BF16)
        nc.sync.dma_start(out=astop_bd, in_=astop_in[:])
        ones_f32 = consts.tile([P, G], F32)
        nc.sync.dma_start(out=ones_f32, in_=onesf_in[:])

        states = []
        for g in range(NGRP):
            st = state_pool.tile([P, SW], BF16, tag=f"st{g}")
            nc.gpsimd.memset(st, 1.0)
            if g == 0:
                # segment 0: x0 = e_0 * a_start
                nc.gpsimd.tensor_mul(out=st[:, :J], in0=init_st, in1=e0)
            states.append(st)

        for k in range(1, NCH):
            raw = raw_pool.tile([P, NWIN, TC, J], F8, tag="raw")
            nc.sync.dma_start(out=raw, in_=e_in[:, k, :])
            ech = epool.tile([P, NWIN, TC, J], F8, tag=f"ech{k}")
            if k <= 3:
                # early chunks: exp per (col, group-half) so ticks 2..7
                # are not paced by whole-chunk 4us exps
                for c in range(TC):
                    for hlf in range(2):
                        s = hlf * SPG
                        nc.scalar.activation(
                            out=ech[:, s:s + SPG, c, :],
                            in_=raw[:, s:s + SPG, c, :],
                            func=mybir.ActivationFunctionType.Exp,
                            bias=bias_c0, scale=1.0)
            else:
                nc.scalar.activation(
                    out=ech, in_=raw, func=mybir.ActivationFunctionType.Exp,
                    bias=bias_c0, scale=1.0)
            echunks[k] = ech

        # gold: Pool add-tree woven into later ticks (<=2 thunks/tick)
        gacc = consts.tile([P, J], F32)
        nc.vector.memset(gacc, 0.0)
        glvl_a = consts.tile([P, J, GC // 2], F32)
        glvl_b = consts.tile([P, J, GC // 4], F32)
        glvl = [glvl_a, glvl_b]
        gold_thunks = []

        def make_gold_chunk(k):
            def dma():
                graw = raw_pool.tile([P, J, GC], F8, tag="graw")
                nc.sync.dma_start(out=graw, in_=gold_in[:, k, :])
                make_gold_chunk.cur = graw
            gold_thunks.append(dma)
            state = {"n": GC, "li": 0}

            def level(state=state):
                src = make_gold_chunk.cur
                n, li = state["n"], state["li"]
                half = n // 2
                if n > 1:
                    dst = glvl[li % 2]
                    nc.gpsimd.tensor_add(
                        out=dst[:, :, :half], in0=src[:, :, :half],
                        in1=src[:, :, half:n])
                    make_gold_chunk.cur = dst
                    state["n"], state["li"] = half, li + 1
                    if half == 1:
                        nc.gpsimd.tensor_add(
                            out=gacc, in0=gacc,
                            in1=make_gold_chunk.cur[:, :, 0])
            for _ in range(7):
                gold_thunks.append(level)
        for k in range(NGC):
            make_gold_chunk(k)

        w_ln = consts.tile([G, H, J], F32)

        w63_src = None
        for tau in range(NT):
            ech, col = echunks[tau // TC], tau % TC
            if tau == NT - 1:
                w63_src = states[1]   # seg-63's final state (tick 31 junks it)
            new_states = []
            for g in range(NGRP):
                st = states[g]
                stn = state_pool.tile([P, SW], BF16, tag=f"st{g}")
                ps = psum_pool.tile([P, NB, 512], F32, tag=f"ps{g}")
                w0 = SPG * g
                for b in range(NB):
                    nc.tensor.matmul(
                        ps[:, b, :HB], bd, st[:, b * HB:(b + 1) * HB],
                        start=True, stop=True)
                nc.vector.tensor_mul(
                    out=stn, in0=ps[:, :, :HB],
                    in1=ech[:, w0:w0 + SPG, col, :])
                new_states.append(stn)
            states = new_states

            if tau >= GOLD_T0:
                for _ in range(2):
                    if gold_thunks:
                        gold_thunks.pop(0)()

        # tail: per-bank column-sums/Lns so group 0's tree overlaps
        # group 1's Lns; the astop dot reads the saved tick-30 state.
        cs0 = psum_pool.tile([G, NB, 512], F32, tag="ps0")
        for b in range(NB):
            nc.tensor.matmul(cs0[:, b, :HB], ones_bd,
                             states[0][:, b * HB:(b + 1) * HB],
                             start=True, stop=True)
        nc.scalar.activation(
            out=w_ln[:, :SPG, :], in_=cs0[:, :, :HB],
            func=mybir.ActivationFunctionType.Ln)
        n = SPG
        while n > 1:
            half = n // 2
            nc.gpsimd.tensor_add(
                out=w_ln[:, :half, :], in0=w_ln[:, :half, :],
                in1=w_ln[:, half:n, :])
            n = half

        cs1 = psum_pool.tile([G, NB, 512], F32, tag="ps1")
        for b in range(NB):
            nc.tensor.matmul(cs1[:, b, :HB], ones_bd,
                             states[1][:, b * HB:(b + 1) * HB],
                             start=True, stop=True)
        nc.tensor.matmul(cs1[:, 2, HB:HB + J], astop_bd,
                         w63_src[:, SW - J:], start=True, stop=True)
        nc.tensor.matmul(cs1[:, 3, HB:HB + J], ones_f32, gacc,
                         start=True, stop=True)
        nc.scalar.activation(
            out=w_ln[:, SPG:, :], in_=cs1[:, :, :HB],
            func=mybir.ActivationFunctionType.Ln)
        nll = small_pool.tile([G, J], F32, tag="nll")
        nc.scalar.activation(
            out=nll, in_=cs1[:, 2, HB:HB + J],
            func=mybir.ActivationFunctionType.Ln)

        # group-1 tree, staggered: bank pairs as their Lns land
        nc.gpsimd.tensor_add(
            out=w_ln[:, SPG:SPG + WB, :], in0=w_ln[:, SPG:SPG + WB, :],
            in1=w_ln[:, SPG + WB:SPG + 2 * WB, :])
        nc.gpsimd.memset(w_ln[:, H - 1, :], 0.0)   # exclude c=H-1
        nc.gpsimd.tensor_add(
            out=w_ln[:, SPG + 2 * WB:SPG + 3 * WB, :],
            in0=w_ln[:, SPG + 2 * WB:SPG + 3 * WB, :],
            in1=w_ln[:, SPG + 3 * WB:SPG + 4 * WB, :])
        nc.gpsimd.tensor_add(
            out=w_ln[:, SPG:SPG + WB, :], in0=w_ln[:, SPG:SPG + WB, :],
            in1=w_ln[:, SPG + 2 * WB:SPG + 3 * WB, :])
        n = WB
        while n > 1:
            half = n // 2
            nc.gpsimd.tensor_add(
                out=w_ln[:, SPG:SPG + half, :], in0=w_ln[:, SPG:SPG + half, :],
                in1=w_ln[:, SPG + half:SPG + n, :])
            n = half
        nc.vector.tensor_add(out=nll, in0=nll, in1=w_ln[:, 0, :])
        nc.vector.tensor_add(out=nll, in0=nll, in1=w_ln[:, SPG, :])
        nc.vector.tensor_sub(out=nll, in0=nll, in1=cs1[:, 3, HB:HB + J])
        nc.vector.tensor_scalar_add(
            out=nll, in0=nll, scalar1=C0 * float(T) - (H - 1) * LN9)
        nc.sync.dma_start(out=out_ext[:], in_=nll)

    nc.finalize()
    return nc


def _host_prep(feats, tags, transitions):
    """Per-core input maps. Pure layout/gather/dtype staging; the only host
    arithmetic is O(K^2) on the 11x11 transition matrix."""
    import ml_dtypes
    f8 = ml_dtypes.float8_e4m3fn
    bf16 = ml_dtypes.bfloat16
    feats = np.asarray(feats, dtype=np.float32)
    tags = np.asarray(tags).astype(np.int64)
    trans = np.asarray(transitions, dtype=np.float32)

    A = np.exp(trans.astype(np.float64)).astype(np.float32)
    Ab = A[:KT, :KT]
    a_start = A[:KT, START].astype(np.float32)
    a_stop = A[STOP, :KT].astype(np.float32)
    eye = np.eye(G, dtype=np.float32)

    bd = np.zeros((P, P), dtype=bf16)
    bd[:PL, :PL] = np.kron(eye, Ab.T).astype(bf16)
    ones_bd = np.zeros((P, G), dtype=bf16)
    ones_bd[:PL] = np.kron(eye, np.ones((KT, 1), np.float32)).astype(bf16)
    astop_bd = np.zeros((P, G), dtype=bf16)
    astop_bd[:PL] = np.kron(eye, a_stop.reshape(KT, 1)).astype(bf16)

    init = np.zeros((P, J), dtype=np.float32)
    for g in range(G):
        init[g * KT:(g + 1) * KT] = a_start[:, None]
    init = init.astype(bf16)

    nslots = G * J

    in_maps = []
    for c in range(NCORES):
        fb = feats[c * BL:(c + 1) * BL, :, :KT]
        tb = tags[c * BL:(c + 1) * BL]

        # emission tile: tilepos = t-1 for t in [1, T); last pos unused
        emis = np.zeros((nslots, H * L, KT), dtype=np.float32)
        emis[:BL, :T - 1] = fb[:, 1:]
        emis8 = emis.astype(f8)

        main = emis8.reshape(nslots, H, NCH, TC, KT)
        full = main.transpose(0, 2, 1, 3, 4).copy()     # [ns,NCH,NWIN,TC,KT]
        e_part = full.reshape(G, J, NCH, NWIN, TC, KT)
        e_part = e_part.transpose(0, 5, 2, 3, 4, 1)
        e8 = np.zeros((P, NCH, CHUNK_ELEMS), dtype=f8)
        e8[:PL] = e_part.reshape(PL, NCH, CHUNK_ELEMS)
        ch0 = e8[:, 0, :].reshape(P, NWIN, TC, J).transpose(0, 2, 1, 3)
        e8[:, 0, :] = np.ascontiguousarray(ch0).reshape(P, CHUNK_ELEMS)

        # e0: feat_0 per sentence, on its group's 9 tag-partitions
        e0 = np.zeros((nslots, KT), dtype=np.float32)
        e0[:BL] = fb[:, 0]
        e0p = np.zeros((P, J), dtype=f8)
        e0p[:PL] = e0.reshape(G, J, KT).transpose(0, 2, 1).reshape(PL, J)

        tt = np.concatenate(
            [np.full((BL, 1), START, dtype=np.int64), tb], axis=1)
        ts = np.concatenate(
            [tb, np.full((BL, 1), STOP, dtype=np.int64)], axis=1)
        trans_sel = trans[ts, tt].astype(f8)
        feat_sel = np.take_along_axis(
            fb, tb[:, :, None], axis=2)[:, :, 0].astype(f8)
        gvals = np.zeros((nslots, KT * GT), dtype=f8)
        gvals[:BL, :T + 1] = trans_sel
        gvals[:BL, T + 1:2 * T + 1] = feat_sel
        gpart = gvals.reshape(G, J, KT, GT).transpose(0, 2, 3, 1)
        g8 = np.zeros((P, NGC, J * GC), dtype=f8)
        g8[:PL] = gpart.reshape(PL, NGC, GC, J).transpose(
            0, 1, 3, 2).reshape(PL, NGC, J * GC)

        in_maps.append({
            "emis8": e8,
            "e0raw": e0p,
            "gold8": g8,
            "bd_lhst": bd,
            "ones_bd": ones_bd,
            "astop_bd": astop_bd,
            "init_st": init,
            "ones_f32": ones_bd.astype(np.float32),
        })
    return in_maps


LAST_EXEC_NS = None


def kernel(feats, tags, transitions):
    global LAST_EXEC_NS
    in_maps = _host_prep(feats, tags, transitions)
    nc = _build_nc()
    trace = os.environ.get("KERNEL_TRACE") == "1"
    res = None
    for attempt in range(3):
        try:
            res = run_bass_kernel_spmd(
                nc, in_maps, list(range(NCORES)), trace=trace)
            break
        except Exception:
            if attempt == 2:
                raise
            import time as _time
            import jax as _jax
            try:
                _jax.clear_caches()
            except Exception:
                pass
            try:
                _jax.clear_backends()
            except Exception:
                pass
            _time.sleep(5)
    LAST_EXEC_NS = res.exec_time_ns
    outs = []
    for c in range(NCORES):
        nll_parts = np.asarray(res.results[c]["nll"], dtype=np.float32)
        outs.append(nll_parts.reshape(-1)[:BL])
    return np.concatenate(outs).astype(np.float32)


if __name__ == "__main__":
    rng = np.random.default_rng(0)
    feats = rng.standard_normal((B, T, K), dtype=np.float32)
    tags = rng.integers(0, 9, size=(B, T), dtype=np.int64)
    trans = rng.random((K, K), dtype=np.float32)
    trans[START, :] = -10000.0
    trans[:, STOP] = -10000.0
    out = kernel(feats=feats, tags=tags, transitions=trans)
    print(out.shape, out[:4])
